# revision 41
# baseline (speedup 1.0000x reference)
"""MoE MLP (dense routing, all experts) Trainium2 Bass kernel.

Problem: nn_MoEMLP_10763188044537
  S, B, D, E = 257, 64, 768, 8 ; H = 4*D = 3072 ; T = S*B = 16448 tokens.
  y[t] = sum_e softmax(x @ Wr.T + br)[t, e] * (quick_gelu(x[t] @ W1[e].T + b1[e]) @ W2[e].T + b2[e])

Sharding: expert-parallel across 8 NeuronCores. Each core holds one
expert's weights resident in SBUF (bf16), streams the replicated
activations, computes the full router softmax locally (with its own
expert's row permuted to column 0 so the same SPMD instruction stream
works on every core), weights its expert's output by the router prob,
and stores its full [D, T] bf16 partial; the host sums the 8 partials
(the gather/unshard step). v3 change: the on-device ReduceScatter was
removed because ANY collective in the NEFF (network rings active)
combined with the x input-DMA stream statically drops the PE cadence
from 216 ns to 263 ns per 512-col matmul for the whole run (~+450 us,
measured by ablation on HW).

Everything runs in [feature, token] orientation:
  fc1:  hT[h, t]  = W1T.T @ xT   (lhsT = W1T chunk, rhs = xT chunk)
  gelu: hg = quick_gelu(hT + b1) (per-partition bias on ACT)
  fc2:  yT[d, t]  = W2T.T @ hg
  comb: yT * Ppb (router prob broadcast to 128 partitions on Pool)

Compute dtype bf16 (inputs are f32; f32 matmuls run at 1/4 rate on the
PE, bf16 at full rate with f32 PSUM accumulation).

Scheduling notes (all measured on HW; clean-program PE cadence is
216 ns per 512-col bf16 matmul = 0.42 ns/col, stream floor ~2.06 ms):
 - NO COLLECTIVES, and do not touch the DMA queue layout casually: the
   PE drops to a uniform 259-263 ns/matmul for the ENTIRE run (+420 us)
   in a program-static way depending on which queues carry input DMAs
   and whether network rings are configured. The current layout
   (x on sync; w1 prologue on gpsimd/scalar/sync; w2+stores on gpsimd)
   measures 216 ns; adding a collective, or spreading x across
   scalar/gpsimd, measured 259-263 ns.
 - The router block for tile i+1 (6 logit matmuls + tanh-softmax DVE
   chain) issues during tile i's fc2; the [8,ct] reciprocal (3.3 us)
   hides under fc2. Router lives at partition base 0; the prob row is
   broadcast to 128 partitions by gpsimd partition_broadcast (its
   ucode reads partition 0 of a base-0 AP only - a base-32 input
   silently reads the wrong SBUF region on HW).
 - PSUM banks: h(fc1)=4, y(fc2)=2, r(logits+sum)=2 -> exactly 8.
 - W1 loads in two stages (first 512 cols, then the rest) so fc1 of
   tile 0 starts ~25 us; PE warmup matmuls cover the DMA lead-in.
 - Tail is two tiles (320 + 256); the last tile's stores alternate
   sync/gpsimd queues to halve the final drain.
"""

import sys

if "/opt/trn_rl_repo" not in sys.path:
    sys.path.insert(0, "/opt/trn_rl_repo")

import ml_dtypes
import numpy as np

S, B, D, E = 257, 64, 768, 8
H = 4 * D
T = S * B
TT = 512           # token tile (free dim of matmuls)
KD = D // 128      # 6 contraction chunks for fc1 / router
KH = H // 128      # 24 contraction chunks for fc2 (and fc1 out chunks)
JD = D // 128      # 6 output-d chunks
GELU_ALPHA = 1.702
RB = 0             # partition base for the router block (lg/sm/ex/...)
N_WARM = 24

BF16 = ml_dtypes.bfloat16

FUSED_GELU = True


def plan_tiles(n_tok):
    """Token tiles: bulk tiles of TT with a (rem-256, 256) tail split.

    groups/chunks are legacy from the ReduceScatter design and unused.
    """
    tiles = []
    t0 = 0
    rem = n_tok
    while rem > TT + 64:
        tiles.append((t0, TT))
        t0 += TT
        rem -= TT
    if rem > 320:
        tiles.append((t0, rem - 256))
        tiles.append((t0 + rem - 256, 256))
    elif rem > 0:
        tiles.append((t0, rem))

    if tiles and tiles[0][1] == TT:
        tiles[0:1] = [(0, TT // 2), (TT // 2, TT // 2)]

    n = len(tiles)
    n_tail = min(2, n)
    bulk, tail = tiles[: n - n_tail], tiles[n - n_tail :]
    groups = []
    i = 0
    while len(bulk) - i > 4:
        take = min(4, len(bulk) - i - 3)
        groups.append(bulk[i : i + take])
        i += take
    r = len(bulk) - i
    for take in {4: (2, 1, 1), 3: (2, 1), 2: (1, 1), 1: (1,), 0: ()}[r]:
        groups.append(bulk[i : i + take])
        i += take
    if tail:
        groups.append(tail)
    chunks = []
    for grp in groups:
        start = grp[0][0]
        width = sum(ct for _, ct in grp)
        chunks.append((start, width))
    return tiles, groups, chunks


def build_nc(n_tok=T, fused_gelu=FUSED_GELU, ablate=()):
    import concourse.mybir as mybir
    import concourse.tile as tile
    from concourse import bacc

    dt = mybir.dt
    F32, BF = dt.float32, dt.bfloat16
    AF = mybir.ActivationFunctionType
    ALU = mybir.AluOpType

    tiles, groups, chunks = plan_tiles(n_tok)
    n_tiles = len(tiles)

    nc = bacc.Bacc("TRN2", target_bir_lowering=False, debug=False, num_devices=E)

    xT = nc.dram_tensor("xT", [D, n_tok], BF, kind="ExternalInput")
    w1t = nc.dram_tensor("w1t", [D, H], BF, kind="ExternalInput")
    w2t = nc.dram_tensor("w2t", [H, D], BF, kind="ExternalInput")
    wrt = nc.dram_tensor("wrt", [128, KD * E], BF, kind="ExternalInput")
    brc = nc.dram_tensor("brc", [RB + E, 1], F32, kind="ExternalInput")
    b1a = nc.dram_tensor("b1a", [128, KH], F32, kind="ExternalInput")
    b1b = nc.dram_tensor("b1b", [128, KH], F32, kind="ExternalInput")
    b2s = nc.dram_tensor("b2s", [128, JD], F32, kind="ExternalInput")
    # Each core emits its expert's full router-weighted partial [D, T] in
    # bf16; the 8-way sum happens host-side. No collective in the NEFF:
    # measured on HW, ANY collective in the program (network rings active)
    # combined with the x input-DMA stream drops the PE from 216 ns to
    # 263 ns per 512-col matmul for the WHOLE run (+450 us). bf16 partial
    # rounding adds ~0.35% error vs the 2e-2 gate.
    yT_out = nc.dram_tensor("yT_out", [D, n_tok], BF, kind="ExternalOutput")

    with tile.TileContext(nc) as tc:
        with (
            tc.tile_pool(name="sb", bufs=1) as sbp,
            tc.tile_pool(name="wp", bufs=1) as wp,
            tc.tile_pool(name="ps", bufs=1, space="PSUM") as psp,
            tc.tile_pool(name="dram", bufs=1, space="DRAM") as dramp,
        ):
            # ---------------- prologue ----------------
            static_x = {}

            def x_dma(ti):
                t0, ct = tiles[ti]
                if "noxdma" in ablate:
                    if not static_x:
                        for k in range(KD):
                            sx = sbp.tile([128, TT], BF, name=f"sx{k}", tag="x", bufs=KD)
                            nc.vector.memset(sx, 0.25)
                            static_x[k] = sx
                    return [static_x[k][:, :ct] for k in range(KD)]
                xts = []
                for k in range(KD):
                    xt = sbp.tile([128, ct], BF, name=f"mx{k}_{ti}", tag="x", bufs=24)
                    src = xT.ap()[k * 128 : (k + 1) * 128, t0 : t0 + ct]
                    nc.sync.dma_start(xt, src)
                    xts.append(xt)
                return xts

            x_tiles = {0: x_dma(0)}

            # W1 k-chunks spread over four queues, in two stages: the first
            # 512 columns (fc1 m=0..3 of tile 0) land early so the PE can
            # start real work at ~25 us; the rest follows.
            w1_sb = [
                wp.tile([128, H], BF, name=f"w1_sb{k}", tag="w1", bufs=KD)
                for k in range(KD)
            ]
            w1_eng = [nc.gpsimd, nc.scalar, nc.scalar, nc.sync, nc.gpsimd, nc.gpsimd]
            W1A = 512
            for k in range(KD):
                w1_eng[k].dma_start(
                    w1_sb[k][:, 0:W1A], w1t.ap()[k * 128 : (k + 1) * 128, 0:W1A]
                )

            wrt_sb = wp.tile([128, KD * E], BF, name="wrt_sb")
            nc.sync.dma_start(wrt_sb, wrt.ap())
            brc_sb = wp.tile([RB + E, 1], F32, name="brc_sb")
            nc.sync.dma_start(brc_sb, brc.ap())
            b1a_sb = wp.tile([128, KH], F32, name="b1a_sb")
            nc.scalar.dma_start(b1a_sb, b1a.ap())

            w1_engB = [nc.gpsimd, nc.scalar, nc.scalar, nc.scalar, nc.gpsimd, nc.gpsimd]
            for k in range(KD):
                w1_engB[k].dma_start(
                    w1_sb[k][:, W1A:H], w1t.ap()[k * 128 : (k + 1) * 128, W1A:H]
                )
            if not fused_gelu:
                b1b_sb = wp.tile([128, KH], F32, name="b1b_sb")
                nc.scalar.dma_start(b1b_sb, b1b.ap())

            if 1 < n_tiles:
                x_tiles[1] = x_dma(1)

            w2_sb = []
            for k in range(KH):
                w = wp.tile([128, D], BF, name=f"w2_sb{k}", tag="w2", bufs=KH)
                nc.gpsimd.dma_start(w, xap(w2t, k * 128, 128)[:, :])
                w2_sb.append(w)
            b2s_sb = wp.tile([128, JD], F32, name="b2s_sb")
            nc.gpsimd.dma_start(b2s_sb, b2s.ap())

            # PE warmup: dependency-free matmuls (memset data, no DMA) keep
            # the PE busy from ~5us while the first DMAs land, and ramp the
            # clock p-state before real work.
            warm_w = wp.tile([128, 128], BF, name="warm_w")
            nc.vector.memset(warm_w, 0.0)
            warm_x = wp.tile([128, tiles[0][1]], BF, name="warm_x")
            nc.vector.memset(warm_x, 0.0)
            for wi in range(N_WARM):
                wp_ps = psp.tile([128, tiles[0][1]], F32, name=f"warm{wi}", tag="h", bufs=4)
                nc.tensor.matmul(wp_ps, lhsT=warm_w, rhs=warm_x, start=True, stop=True)

            ones_r = wp.tile([RB + E, 1], BF, name="ones_r")
            nc.vector.memset(ones_r, 1.0)





            # ---------------- router block emission ----------------
            # Two stages, both called one phase early (during tile ti-1's
            # fc2) so the probs are ready before tile ti's fc1 ends. The
            # matmul+tanh stage goes at fc2 start; the DVE chain (with its
            # 3.3us [8,ct] reciprocal) is emitted after yw1 so the in-order
            # Vector queue doesn't delay the fc2 prob-weight multiplies.
            def emit_router_mm(ti):
                t0, ct = tiles[ti]
                xts = x_tiles[ti]
                r_ps = psp.tile([128, ct], F32, name=f"r{ti}", tag="r", bufs=2)
                lg = r_ps[RB : RB + E, :]
                for k in range(KD):
                    nc.tensor.matmul(
                        lg,
                        lhsT=wrt_sb[:, k * 8 : (k + 1) * 8],
                        rhs=xts[k],
                        start=(k == 0),
                        stop=(k == KD - 1),
                    )
                # softmax via exp(l) = (1+tanh((l+br)/2)) / (1-tanh((l+br)/2))
                th = sbp.tile([RB + E, ct], F32, name=f"th{ti}", tag="th", bufs=2)
                nc.scalar.activation(
                    th[RB:, :], lg, AF.Tanh, bias=brc_sb[RB:, :], scale=0.5
                )
                return [r_ps, th, None]

            def emit_router_dve(ti, st):
                t0, ct = tiles[ti]
                th = st[1]
                num = sbp.tile([RB + E, ct], F32, name=f"num{ti}", tag="rchain", bufs=6)
                nc.vector.tensor_scalar_add(num[RB:, :], th[RB:, :], 1.0)
                den = sbp.tile([RB + E, ct], F32, name=f"den{ti}", tag="rchain", bufs=6)
                nc.vector.tensor_scalar(
                    den[RB:, :], th[RB:, :], 1.0, -1.0, op0=ALU.subtract, op1=ALU.mult
                )
                rd = sbp.tile([RB + E, ct], F32, name=f"rd{ti}", tag="rchain", bufs=6)
                nc.vector.reciprocal(rd[RB:, :], den[RB:, :])
                ex = sbp.tile([RB + E, ct], BF, name=f"ex{ti}", tag="ex", bufs=2)
                nc.vector.tensor_tensor(ex[RB:, :], num[RB:, :], rd[RB:, :], op=ALU.mult)
                st[2] = ex

            # Emits the softmax sum + prob normalization + broadcast for tile
            # ti (a few fc1 m-chunks in, so the DVE chain has had time). All
            # off the PE: partition add-tree and the 128-row broadcast run on
            # the (otherwise idle) Pool engine.
            def emit_prob(ti, r_ps, ex):
                t0, ct = tiles[ti]
                sm = r_ps[RB : RB + 1, :]
                nc.tensor.matmul(
                    sm, lhsT=ones_r[RB:, :], rhs=ex[RB:, :], start=True, stop=True
                )
                rc = sbp.tile([RB + 1, ct], F32, name=f"rc{ti}", tag="rc", bufs=2)
                nc.vector.reciprocal(rc[RB:, :], sm)
                pp = sbp.tile([RB + 1, ct], BF, name=f"pp{ti}", tag="pp", bufs=2)
                nc.vector.tensor_tensor(
                    pp[RB:, :], ex[RB : RB + 1, :], rc[RB:, :], op=ALU.mult
                )
                # router-prob broadcast to 128 partitions on the (idle) Pool
                # engine; needs a base-0 input AP (ucode reads partition 0).
                ppb = sbp.tile([128, ct], BF, name=f"ppb{ti}", tag="ppb", bufs=2)
                nc.gpsimd.partition_broadcast(ppb, pp[RB : RB + 1, :])
                return ppb

            if "norouter" in ablate:
                r_static = sbp.tile([128, TT], BF, name="r_static")
                nc.vector.memset(r_static, 0.125)

                def emit_router_mm(ti):  # noqa: F811
                    return [None, None, None]

                def emit_router_dve(ti, st):  # noqa: F811
                    pass

            router_state = {0: emit_router_mm(0)}
            emit_router_dve(0, router_state[0])

            # ---------------- main loop ----------------
            for ti, (t0, ct) in enumerate(tiles):
                xts = x_tiles.pop(ti)
                r_ps, _th, ex = router_state.pop(ti)
                if "norouter" in ablate:
                    ppb = r_static[:, :ct]

                # fc1 + gelu
                hgs = []
                for m in range(KH):
                    hp = psp.tile([128, ct], F32, name=f"hp{ti}_{m}", tag="h", bufs=4)
                    for k in range(KD):
                        nc.tensor.matmul(
                            hp,
                            lhsT=w1_sb[k][:, m * 128 : (m + 1) * 128],
                            rhs=xts[k],
                            start=(k == 0),
                            stop=(k == KD - 1),
                        )
                    hg = sbp.tile(
                        [128, ct], BF, name=f"hg{ti}_{m}", tag="hg", bufs=48
                    )
                    if fused_gelu:
                        nc.scalar.activation(
                            hg,
                            hp,
                            AF.Gelu_apprx_sigmoid,
                            bias=b1a_sb[:, m : m + 1],
                            scale=1.0,
                        )
                    else:
                        sg = sbp.tile([128, ct], F32, name=f"sg{ti}_{m}", tag="sg", bufs=3)
                        nc.scalar.activation(
                            sg,
                            hp,
                            AF.Sigmoid,
                            bias=b1b_sb[:, m : m + 1],
                            scale=GELU_ALPHA,
                        )
                        zz = sbp.tile([128, ct], F32, name=f"zz{ti}_{m}", tag="zz", bufs=3)
                        nc.vector.tensor_scalar_add(zz, hp, b1a_sb[:, m : m + 1])
                        nc.vector.tensor_tensor(hg, zz, sg, op=ALU.mult)
                    hgs.append(hg)

                    if m == 4 and "norouter" not in ablate:
                        ppb = emit_prob(ti, r_ps, ex)

                # next tile's x (two ahead) and router logits (one ahead)
                if ti + 2 < n_tiles and (ti + 2) not in x_tiles:
                    x_tiles[ti + 2] = x_dma(ti + 2)
                if ti + 1 < n_tiles:
                    if (ti + 1) not in x_tiles:
                        x_tiles[ti + 1] = x_dma(ti + 1)
                    router_state[ti + 1] = emit_router_mm(ti + 1)

                # fc2 + bias + prob-weight + store
                for j in range(JD):
                    yp = psp.tile([128, ct], F32, name=f"yp{ti}_{j}", tag="y", bufs=2)
                    for k in range(KH):
                        nc.tensor.matmul(
                            yp,
                            lhsT=w2_sb[k][:, j * 128 : (j + 1) * 128],
                            rhs=hgs[k],
                            start=(k == 0),
                            stop=(k == KH - 1),
                        )
                    yb = sbp.tile([128, ct], F32, name=f"yb{ti}_{j}", tag="yb", bufs=3)
                    nc.scalar.activation(yb, yp, AF.Identity, bias=b2s_sb[:, j : j + 1])
                    yw = sbp.tile([128, ct], BF, name=f"yw{ti}_{j}", tag="yw", bufs=4)
                    nc.vector.tensor_tensor(yw, yb, ppb, op=ALU.mult)
                    if "nostore" not in ablate:
                        # Split the final tile's stores across two queues so
                        # the end-of-program store drain halves.
                        st_eng = nc.sync if ti == n_tiles - 1 and j % 2 else nc.gpsimd
                        st_eng.dma_start(
                            yT_out.ap()[j * 128 : (j + 1) * 128, t0 : t0 + ct], yw
                        )
                    if j == 1 and ti + 1 < n_tiles:
                        emit_router_dve(ti + 1, router_state[ti + 1])

            if "nostore" in ablate:
                zsb = sbp.tile([128, TT], BF, name="zsb")
                nc.vector.memset(zsb, 0.0)
                for j in range(JD):
                    for t0, ct in tiles:
                        nc.sync.dma_start(
                            yT_out.ap()[j * 128 : (j + 1) * 128, t0 : t0 + ct],
                            zsb[:, :ct],
                        )

    nc.compile()
    return nc


def xap(handle, row0, nrows):
    """Row-slice helper for 2D DRAM tensors."""
    return handle.ap()[row0 : row0 + nrows, :]


def prep_inputs(x, W1, b1, W2, b2, Wr, br):
    """Host-side shard prep. Returns in_maps for the 8 cores."""
    x = np.asarray(x, dtype=np.float32)
    s, b, d = x.shape
    xf = x.reshape(s * b, d)
    xT_bf = np.ascontiguousarray(xf.T).astype(BF16)  # [D, T]

    W1 = np.asarray(W1, dtype=np.float32)
    W2 = np.asarray(W2, dtype=np.float32)
    b1 = np.asarray(b1, dtype=np.float32)
    b2 = np.asarray(b2, dtype=np.float32)
    Wr = np.asarray(Wr, dtype=np.float32)
    br = np.asarray(br, dtype=np.float32)

    in_maps = []
    for e in range(E):
        perm = [e] + [i for i in range(E) if i != e]
        w1t = np.ascontiguousarray(W1[e].T).astype(BF16)  # [D, H]
        w2t = np.ascontiguousarray(W2[e].T).astype(BF16)  # [H, D]
        wrt_p = np.ascontiguousarray(Wr[perm].T)  # [D, E]
        wrt = np.ascontiguousarray(
            wrt_p.reshape(KD, 128, E).transpose(1, 0, 2).reshape(128, KD * E)
        ).astype(BF16)
        brc = np.zeros((RB + E, 1), dtype=np.float32)
        brc[RB:, 0] = 0.5 * br[perm]
        b1a = np.ascontiguousarray(b1[e].reshape(KH, 128).T)  # [128, KH] f32
        b1b = np.ascontiguousarray(GELU_ALPHA * b1a)
        b2sa = np.ascontiguousarray(b2[e].reshape(JD, 128).T)  # [128, JD] f32
        in_maps.append(
            {
                "xT": xT_bf,
                "w1t": w1t,
                "w2t": w2t,
                "wrt": wrt,
                "brc": brc,
                "b1a": b1a,
                "b1b": b1b,
                "b2s": b2sa,
            }
        )
    return in_maps


def assemble_output(per_core_yT, s, b):
    """Sum the 8 per-expert partials (host-side combine), then [D,T] -> [s,b,D]."""
    yT = np.asarray(per_core_yT[0], dtype=np.float32)
    for p in per_core_yT[1:]:
        yT += np.asarray(p, dtype=np.float32)
    return np.ascontiguousarray(yT.T).reshape(s, b, D)


_NC_CACHE = {}


def get_nc(n_tok=T, fused_gelu=FUSED_GELU):
    key = (n_tok, fused_gelu)
    if key not in _NC_CACHE:
        _NC_CACHE[key] = build_nc(n_tok, fused_gelu)
    return _NC_CACHE[key]


def _axon_reset():
    """Best-effort reset of the axon-tunneled NeuronCores.

    The tunnel occasionally wedges (NRT_EXEC_UNIT_UNRECOVERABLE / NaN
    output on the run right after loading a fresh NEFF); a reset + retry
    has always recovered it.
    """
    try:
        import ctypes

        lib = ctypes.CDLL("/opt/axon/libaxon_pjrt.so")
        if hasattr(lib, "axon_reset"):
            lib.axon_reset.restype = ctypes.c_int64
            lib.axon_reset()
    except Exception:
        pass


def kernel(x, W1, b1, W2, b2, Wr, br, trace=False):
    from concourse.bass_utils import run_bass_kernel_spmd

    x = np.asarray(x, dtype=np.float32)
    s, b, d = x.shape
    nc = get_nc(n_tok=s * b)
    in_maps = prep_inputs(x, W1, b1, W2, b2, Wr, br)

    last_exc = None
    for attempt in range(3):
        try:
            res = run_bass_kernel_spmd(
                nc, in_maps, core_ids=list(range(E)), trace=trace
            )
            out = assemble_output(
                [res.results[e]["yT_out"] for e in range(E)], s, b
            )
            if np.isfinite(out).all():
                if trace:
                    kernel.last_result = res
                return out
            print(f"kernel: non-finite output on attempt {attempt}; resetting")
        except Exception as exc:  # transient tunnel/device wedge
            last_exc = exc
            print(f"kernel: attempt {attempt} failed ({exc!r}); resetting")
        _axon_reset()
    if last_exc is not None:
        raise last_exc
    raise RuntimeError("kernel produced non-finite output on all attempts")



# revision 42
# speedup vs baseline: 1.0029x; 1.0029x over previous
"""MoE MLP (dense routing, all experts) Trainium2 Bass kernel.

Problem: nn_MoEMLP_10763188044537
  S, B, D, E = 257, 64, 768, 8 ; H = 4*D = 3072 ; T = S*B = 16448 tokens.
  y[t] = sum_e softmax(x @ Wr.T + br)[t, e] * (quick_gelu(x[t] @ W1[e].T + b1[e]) @ W2[e].T + b2[e])

Sharding: expert-parallel across 8 NeuronCores. Each core holds one
expert's weights resident in SBUF (bf16), streams the replicated
activations, computes the full router softmax locally (with its own
expert's row permuted to column 0 so the same SPMD instruction stream
works on every core), weights its expert's output by the router prob,
and stores its full [D, T] bf16 partial; the host sums the 8 partials
(the gather/unshard step). v3 change: the on-device ReduceScatter was
removed because ANY collective in the NEFF (network rings active)
combined with the x input-DMA stream statically drops the PE cadence
from 216 ns to 263 ns per 512-col matmul for the whole run (~+450 us,
measured by ablation on HW).

Everything runs in [feature, token] orientation:
  fc1:  hT[h, t]  = W1T.T @ xT   (lhsT = W1T chunk, rhs = xT chunk)
  gelu: hg = quick_gelu(hT + b1) (per-partition bias on ACT)
  fc2:  yT[d, t]  = W2T.T @ hg
  comb: yT * Ppb (router prob broadcast to 128 partitions on Pool)

Compute dtype bf16 (inputs are f32; f32 matmuls run at 1/4 rate on the
PE, bf16 at full rate with f32 PSUM accumulation).

Scheduling notes (all measured on HW; clean-program PE cadence is
216 ns per 512-col bf16 matmul = 0.42 ns/col, stream floor ~2.06 ms):
 - NO COLLECTIVES, and do not touch the DMA queue layout casually: the
   PE drops to a uniform 259-263 ns/matmul for the ENTIRE run (+420 us)
   in a program-static way depending on which queues carry input DMAs
   and whether network rings are configured. The current layout
   (x on sync; w1 prologue on gpsimd/scalar/sync; w2+stores on gpsimd)
   measures 216 ns; adding a collective, or spreading x across
   scalar/gpsimd, measured 259-263 ns.
 - The router block for tile i+1 (6 logit matmuls + tanh-softmax DVE
   chain) issues during tile i's fc2; the [8,ct] reciprocal (3.3 us)
   hides under fc2. Router lives at partition base 0; the prob row is
   broadcast to 128 partitions by gpsimd partition_broadcast (its
   ucode reads partition 0 of a base-0 AP only - a base-32 input
   silently reads the wrong SBUF region on HW).
 - PSUM banks: h(fc1)=4, y(fc2)=2, r(logits+sum)=2 -> exactly 8.
 - W1 loads in two stages (first 512 cols, then the rest) so fc1 of
   tile 0 starts ~25 us; PE warmup matmuls cover the DMA lead-in.
 - Tail is two tiles (320 + 256); the last tile's stores alternate
   sync/gpsimd queues to halve the final drain.
"""

import sys

if "/opt/trn_rl_repo" not in sys.path:
    sys.path.insert(0, "/opt/trn_rl_repo")

import ml_dtypes
import numpy as np

S, B, D, E = 257, 64, 768, 8
H = 4 * D
T = S * B
TT = 512           # token tile (free dim of matmuls)
KD = D // 128      # 6 contraction chunks for fc1 / router
KH = H // 128      # 24 contraction chunks for fc2 (and fc1 out chunks)
JD = D // 128      # 6 output-d chunks
GELU_ALPHA = 1.702
RB = 0             # partition base for the router block (lg/sm/ex/...)
N_WARM = 24

BF16 = ml_dtypes.bfloat16

FUSED_GELU = True


def plan_tiles(n_tok):
    """Token tiles: bulk tiles of TT with a (rem-256, 256) tail split.

    groups/chunks are legacy from the ReduceScatter design and unused.
    """
    tiles = []
    t0 = 0
    rem = n_tok
    while rem > TT + 64:
        tiles.append((t0, TT))
        t0 += TT
        rem -= TT
    if rem > 320:
        tiles.append((t0, rem - 256))
        tiles.append((t0 + rem - 256, 256))
    elif rem > 0:
        tiles.append((t0, rem))

    n = len(tiles)
    n_tail = min(2, n)
    bulk, tail = tiles[: n - n_tail], tiles[n - n_tail :]
    groups = []
    i = 0
    while len(bulk) - i > 4:
        take = min(4, len(bulk) - i - 3)
        groups.append(bulk[i : i + take])
        i += take
    r = len(bulk) - i
    for take in {4: (2, 1, 1), 3: (2, 1), 2: (1, 1), 1: (1,), 0: ()}[r]:
        groups.append(bulk[i : i + take])
        i += take
    if tail:
        groups.append(tail)
    chunks = []
    for grp in groups:
        start = grp[0][0]
        width = sum(ct for _, ct in grp)
        chunks.append((start, width))
    return tiles, groups, chunks


def build_nc(n_tok=T, fused_gelu=FUSED_GELU, ablate=()):
    import concourse.mybir as mybir
    import concourse.tile as tile
    from concourse import bacc

    dt = mybir.dt
    F32, BF = dt.float32, dt.bfloat16
    AF = mybir.ActivationFunctionType
    ALU = mybir.AluOpType

    tiles, groups, chunks = plan_tiles(n_tok)
    n_tiles = len(tiles)

    nc = bacc.Bacc("TRN2", target_bir_lowering=False, debug=False, num_devices=E)

    xT = nc.dram_tensor("xT", [D, n_tok], BF, kind="ExternalInput")
    w1t = nc.dram_tensor("w1t", [D, H], BF, kind="ExternalInput")
    w2t = nc.dram_tensor("w2t", [H, D], BF, kind="ExternalInput")
    wrt = nc.dram_tensor("wrt", [128, KD * E], BF, kind="ExternalInput")
    brc = nc.dram_tensor("brc", [RB + E, 1], F32, kind="ExternalInput")
    b1a = nc.dram_tensor("b1a", [128, KH], F32, kind="ExternalInput")
    b1b = nc.dram_tensor("b1b", [128, KH], F32, kind="ExternalInput")
    b2s = nc.dram_tensor("b2s", [128, JD], F32, kind="ExternalInput")
    # Each core emits its expert's full router-weighted partial [D, T] in
    # bf16; the 8-way sum happens host-side. No collective in the NEFF:
    # measured on HW, ANY collective in the program (network rings active)
    # combined with the x input-DMA stream drops the PE from 216 ns to
    # 263 ns per 512-col matmul for the WHOLE run (+450 us). bf16 partial
    # rounding adds ~0.35% error vs the 2e-2 gate.
    yT_out = nc.dram_tensor("yT_out", [D, n_tok], BF, kind="ExternalOutput")

    with tile.TileContext(nc) as tc:
        with (
            tc.tile_pool(name="sb", bufs=1) as sbp,
            tc.tile_pool(name="wp", bufs=1) as wp,
            tc.tile_pool(name="ps", bufs=1, space="PSUM") as psp,
            tc.tile_pool(name="dram", bufs=1, space="DRAM") as dramp,
        ):
            # ---------------- prologue ----------------
            static_x = {}

            def x_dma(ti):
                t0, ct = tiles[ti]
                if "noxdma" in ablate:
                    if not static_x:
                        for k in range(KD):
                            sx = sbp.tile([128, TT], BF, name=f"sx{k}", tag="x", bufs=KD)
                            nc.vector.memset(sx, 0.25)
                            static_x[k] = sx
                    return [static_x[k][:, :ct] for k in range(KD)]
                xts = []
                for k in range(KD):
                    xt = sbp.tile([128, ct], BF, name=f"mx{k}_{ti}", tag="x", bufs=24)
                    src = xT.ap()[k * 128 : (k + 1) * 128, t0 : t0 + ct]
                    nc.sync.dma_start(xt, src)
                    xts.append(xt)
                return xts

            x_tiles = {0: x_dma(0)}

            # W1 k-chunks spread over four queues, in two stages: the first
            # 512 columns (fc1 m=0..3 of tile 0) land early so the PE can
            # start real work at ~25 us; the rest follows.
            w1_sb = [
                wp.tile([128, H], BF, name=f"w1_sb{k}", tag="w1", bufs=KD)
                for k in range(KD)
            ]
            w1_eng = [nc.gpsimd, nc.scalar, nc.scalar, nc.sync, nc.gpsimd, nc.gpsimd]
            W1A = 512
            for k in range(KD):
                w1_eng[k].dma_start(
                    w1_sb[k][:, 0:W1A], w1t.ap()[k * 128 : (k + 1) * 128, 0:W1A]
                )

            wrt_sb = wp.tile([128, KD * E], BF, name="wrt_sb")
            nc.sync.dma_start(wrt_sb, wrt.ap())
            brc_sb = wp.tile([RB + E, 1], F32, name="brc_sb")
            nc.sync.dma_start(brc_sb, brc.ap())
            b1a_sb = wp.tile([128, KH], F32, name="b1a_sb")
            nc.scalar.dma_start(b1a_sb, b1a.ap())

            for k in range(KD):
                w1_eng[k].dma_start(
                    w1_sb[k][:, W1A:H], w1t.ap()[k * 128 : (k + 1) * 128, W1A:H]
                )
            if not fused_gelu:
                b1b_sb = wp.tile([128, KH], F32, name="b1b_sb")
                nc.scalar.dma_start(b1b_sb, b1b.ap())

            if 1 < n_tiles:
                x_tiles[1] = x_dma(1)

            w2_sb = []
            for k in range(KH):
                w = wp.tile([128, D], BF, name=f"w2_sb{k}", tag="w2", bufs=KH)
                nc.gpsimd.dma_start(w, xap(w2t, k * 128, 128)[:, :])
                w2_sb.append(w)
            b2s_sb = wp.tile([128, JD], F32, name="b2s_sb")
            nc.gpsimd.dma_start(b2s_sb, b2s.ap())

            # PE warmup: dependency-free matmuls (memset data, no DMA) keep
            # the PE busy from ~5us while the first DMAs land, and ramp the
            # clock p-state before real work.
            warm_w = wp.tile([128, 128], BF, name="warm_w")
            nc.vector.memset(warm_w, 0.0)
            warm_x = wp.tile([128, tiles[0][1]], BF, name="warm_x")
            nc.vector.memset(warm_x, 0.0)
            for wi in range(N_WARM):
                wp_ps = psp.tile([128, tiles[0][1]], F32, name=f"warm{wi}", tag="h", bufs=4)
                nc.tensor.matmul(wp_ps, lhsT=warm_w, rhs=warm_x, start=True, stop=True)

            ones_r = wp.tile([RB + E, 1], BF, name="ones_r")
            nc.vector.memset(ones_r, 1.0)





            # ---------------- router block emission ----------------
            # Two stages, both called one phase early (during tile ti-1's
            # fc2) so the probs are ready before tile ti's fc1 ends. The
            # matmul+tanh stage goes at fc2 start; the DVE chain (with its
            # 3.3us [8,ct] reciprocal) is emitted after yw1 so the in-order
            # Vector queue doesn't delay the fc2 prob-weight multiplies.
            def emit_router_mm(ti):
                t0, ct = tiles[ti]
                xts = x_tiles[ti]
                r_ps = psp.tile([128, ct], F32, name=f"r{ti}", tag="r", bufs=2)
                lg = r_ps[RB : RB + E, :]
                for k in range(KD):
                    nc.tensor.matmul(
                        lg,
                        lhsT=wrt_sb[:, k * 8 : (k + 1) * 8],
                        rhs=xts[k],
                        start=(k == 0),
                        stop=(k == KD - 1),
                    )
                # softmax via exp(l) = (1+tanh((l+br)/2)) / (1-tanh((l+br)/2))
                th = sbp.tile([RB + E, ct], F32, name=f"th{ti}", tag="th", bufs=2)
                nc.scalar.activation(
                    th[RB:, :], lg, AF.Tanh, bias=brc_sb[RB:, :], scale=0.5
                )
                return [r_ps, th, None]

            def emit_router_dve(ti, st):
                t0, ct = tiles[ti]
                th = st[1]
                num = sbp.tile([RB + E, ct], F32, name=f"num{ti}", tag="rchain", bufs=6)
                nc.vector.tensor_scalar_add(num[RB:, :], th[RB:, :], 1.0)
                den = sbp.tile([RB + E, ct], F32, name=f"den{ti}", tag="rchain", bufs=6)
                nc.vector.tensor_scalar(
                    den[RB:, :], th[RB:, :], 1.0, -1.0, op0=ALU.subtract, op1=ALU.mult
                )
                rd = sbp.tile([RB + E, ct], F32, name=f"rd{ti}", tag="rchain", bufs=6)
                nc.vector.reciprocal(rd[RB:, :], den[RB:, :])
                ex = sbp.tile([RB + E, ct], BF, name=f"ex{ti}", tag="ex", bufs=2)
                nc.vector.tensor_tensor(ex[RB:, :], num[RB:, :], rd[RB:, :], op=ALU.mult)
                st[2] = ex

            # Emits the softmax sum + prob normalization + broadcast for tile
            # ti (a few fc1 m-chunks in, so the DVE chain has had time). All
            # off the PE: partition add-tree and the 128-row broadcast run on
            # the (otherwise idle) Pool engine.
            def emit_prob(ti, r_ps, ex):
                t0, ct = tiles[ti]
                sm = r_ps[RB : RB + 1, :]
                nc.tensor.matmul(
                    sm, lhsT=ones_r[RB:, :], rhs=ex[RB:, :], start=True, stop=True
                )
                rc = sbp.tile([RB + 1, ct], F32, name=f"rc{ti}", tag="rc", bufs=2)
                nc.vector.reciprocal(rc[RB:, :], sm)
                pp = sbp.tile([RB + 1, ct], BF, name=f"pp{ti}", tag="pp", bufs=2)
                nc.vector.tensor_tensor(
                    pp[RB:, :], ex[RB : RB + 1, :], rc[RB:, :], op=ALU.mult
                )
                # router-prob broadcast to 128 partitions on the (idle) Pool
                # engine; needs a base-0 input AP (ucode reads partition 0).
                ppb = sbp.tile([128, ct], BF, name=f"ppb{ti}", tag="ppb", bufs=2)
                nc.gpsimd.partition_broadcast(ppb, pp[RB : RB + 1, :])
                return ppb

            if "norouter" in ablate:
                r_static = sbp.tile([128, TT], BF, name="r_static")
                nc.vector.memset(r_static, 0.125)

                def emit_router_mm(ti):  # noqa: F811
                    return [None, None, None]

                def emit_router_dve(ti, st):  # noqa: F811
                    pass

            router_state = {0: emit_router_mm(0)}
            emit_router_dve(0, router_state[0])

            # ---------------- main loop ----------------
            for ti, (t0, ct) in enumerate(tiles):
                xts = x_tiles.pop(ti)
                r_ps, _th, ex = router_state.pop(ti)
                if "norouter" in ablate:
                    ppb = r_static[:, :ct]

                # fc1 + gelu
                hgs = []
                for m in range(KH):
                    hp = psp.tile([128, ct], F32, name=f"hp{ti}_{m}", tag="h", bufs=4)
                    for k in range(KD):
                        nc.tensor.matmul(
                            hp,
                            lhsT=w1_sb[k][:, m * 128 : (m + 1) * 128],
                            rhs=xts[k],
                            start=(k == 0),
                            stop=(k == KD - 1),
                        )
                    hg = sbp.tile(
                        [128, ct], BF, name=f"hg{ti}_{m}", tag="hg", bufs=48
                    )
                    if fused_gelu:
                        nc.scalar.activation(
                            hg,
                            hp,
                            AF.Gelu_apprx_sigmoid,
                            bias=b1a_sb[:, m : m + 1],
                            scale=1.0,
                        )
                    else:
                        sg = sbp.tile([128, ct], F32, name=f"sg{ti}_{m}", tag="sg", bufs=3)
                        nc.scalar.activation(
                            sg,
                            hp,
                            AF.Sigmoid,
                            bias=b1b_sb[:, m : m + 1],
                            scale=GELU_ALPHA,
                        )
                        zz = sbp.tile([128, ct], F32, name=f"zz{ti}_{m}", tag="zz", bufs=3)
                        nc.vector.tensor_scalar_add(zz, hp, b1a_sb[:, m : m + 1])
                        nc.vector.tensor_tensor(hg, zz, sg, op=ALU.mult)
                    hgs.append(hg)

                    if m == 4 and "norouter" not in ablate:
                        ppb = emit_prob(ti, r_ps, ex)

                # next tile's x (two ahead) and router logits (one ahead)
                if ti + 2 < n_tiles and (ti + 2) not in x_tiles:
                    x_tiles[ti + 2] = x_dma(ti + 2)
                if ti + 1 < n_tiles:
                    if (ti + 1) not in x_tiles:
                        x_tiles[ti + 1] = x_dma(ti + 1)
                    router_state[ti + 1] = emit_router_mm(ti + 1)

                # fc2 + bias + prob-weight + store
                for j in range(JD):
                    yp = psp.tile([128, ct], F32, name=f"yp{ti}_{j}", tag="y", bufs=2)
                    for k in range(KH):
                        nc.tensor.matmul(
                            yp,
                            lhsT=w2_sb[k][:, j * 128 : (j + 1) * 128],
                            rhs=hgs[k],
                            start=(k == 0),
                            stop=(k == KH - 1),
                        )
                    yb = sbp.tile([128, ct], F32, name=f"yb{ti}_{j}", tag="yb", bufs=3)
                    nc.scalar.activation(yb, yp, AF.Identity, bias=b2s_sb[:, j : j + 1])
                    yw = sbp.tile([128, ct], BF, name=f"yw{ti}_{j}", tag="yw", bufs=4)
                    nc.vector.tensor_tensor(yw, yb, ppb, op=ALU.mult)
                    if "nostore" not in ablate:
                        # Split the final tile's stores across two queues so
                        # the end-of-program store drain halves.
                        st_eng = nc.sync if ti == n_tiles - 1 and j % 2 else nc.gpsimd
                        st_eng.dma_start(
                            yT_out.ap()[j * 128 : (j + 1) * 128, t0 : t0 + ct], yw
                        )
                    if j == 1 and ti + 1 < n_tiles:
                        emit_router_dve(ti + 1, router_state[ti + 1])

            if "nostore" in ablate:
                zsb = sbp.tile([128, TT], BF, name="zsb")
                nc.vector.memset(zsb, 0.0)
                for j in range(JD):
                    for t0, ct in tiles:
                        nc.sync.dma_start(
                            yT_out.ap()[j * 128 : (j + 1) * 128, t0 : t0 + ct],
                            zsb[:, :ct],
                        )

    nc.compile()
    return nc


def xap(handle, row0, nrows):
    """Row-slice helper for 2D DRAM tensors."""
    return handle.ap()[row0 : row0 + nrows, :]


def prep_inputs(x, W1, b1, W2, b2, Wr, br):
    """Host-side shard prep. Returns in_maps for the 8 cores."""
    x = np.asarray(x, dtype=np.float32)
    s, b, d = x.shape
    xf = x.reshape(s * b, d)
    xT_bf = np.ascontiguousarray(xf.T).astype(BF16)  # [D, T]

    W1 = np.asarray(W1, dtype=np.float32)
    W2 = np.asarray(W2, dtype=np.float32)
    b1 = np.asarray(b1, dtype=np.float32)
    b2 = np.asarray(b2, dtype=np.float32)
    Wr = np.asarray(Wr, dtype=np.float32)
    br = np.asarray(br, dtype=np.float32)

    in_maps = []
    for e in range(E):
        perm = [e] + [i for i in range(E) if i != e]
        w1t = np.ascontiguousarray(W1[e].T).astype(BF16)  # [D, H]
        w2t = np.ascontiguousarray(W2[e].T).astype(BF16)  # [H, D]
        wrt_p = np.ascontiguousarray(Wr[perm].T)  # [D, E]
        wrt = np.ascontiguousarray(
            wrt_p.reshape(KD, 128, E).transpose(1, 0, 2).reshape(128, KD * E)
        ).astype(BF16)
        brc = np.zeros((RB + E, 1), dtype=np.float32)
        brc[RB:, 0] = 0.5 * br[perm]
        b1a = np.ascontiguousarray(b1[e].reshape(KH, 128).T)  # [128, KH] f32
        b1b = np.ascontiguousarray(GELU_ALPHA * b1a)
        b2sa = np.ascontiguousarray(b2[e].reshape(JD, 128).T)  # [128, JD] f32
        in_maps.append(
            {
                "xT": xT_bf,
                "w1t": w1t,
                "w2t": w2t,
                "wrt": wrt,
                "brc": brc,
                "b1a": b1a,
                "b1b": b1b,
                "b2s": b2sa,
            }
        )
    return in_maps


def assemble_output(per_core_yT, s, b):
    """Sum the 8 per-expert partials (host-side combine), then [D,T] -> [s,b,D]."""
    yT = np.asarray(per_core_yT[0], dtype=np.float32)
    for p in per_core_yT[1:]:
        yT += np.asarray(p, dtype=np.float32)
    return np.ascontiguousarray(yT.T).reshape(s, b, D)


_NC_CACHE = {}


def get_nc(n_tok=T, fused_gelu=FUSED_GELU):
    key = (n_tok, fused_gelu)
    if key not in _NC_CACHE:
        _NC_CACHE[key] = build_nc(n_tok, fused_gelu)
    return _NC_CACHE[key]


def _axon_reset():
    """Best-effort reset of the axon-tunneled NeuronCores.

    The tunnel occasionally wedges (NRT_EXEC_UNIT_UNRECOVERABLE / NaN
    output on the run right after loading a fresh NEFF); a reset + retry
    has always recovered it.
    """
    try:
        import ctypes

        lib = ctypes.CDLL("/opt/axon/libaxon_pjrt.so")
        if hasattr(lib, "axon_reset"):
            lib.axon_reset.restype = ctypes.c_int64
            lib.axon_reset()
    except Exception:
        pass


def kernel(x, W1, b1, W2, b2, Wr, br, trace=False):
    from concourse.bass_utils import run_bass_kernel_spmd

    x = np.asarray(x, dtype=np.float32)
    s, b, d = x.shape
    nc = get_nc(n_tok=s * b)
    in_maps = prep_inputs(x, W1, b1, W2, b2, Wr, br)

    last_exc = None
    for attempt in range(3):
        try:
            res = run_bass_kernel_spmd(
                nc, in_maps, core_ids=list(range(E)), trace=trace
            )
            out = assemble_output(
                [res.results[e]["yT_out"] for e in range(E)], s, b
            )
            if np.isfinite(out).all():
                if trace:
                    kernel.last_result = res
                return out
            print(f"kernel: non-finite output on attempt {attempt}; resetting")
        except Exception as exc:  # transient tunnel/device wedge
            last_exc = exc
            print(f"kernel: attempt {attempt} failed ({exc!r}); resetting")
        _axon_reset()
    if last_exc is not None:
        raise last_exc
    raise RuntimeError("kernel produced non-finite output on all attempts")

